# revision 8
# baseline (speedup 1.0000x reference)
"""Trainium2 Bass kernel for a pre-LN multi-head attention block.

Computes, for x [4, 2048, 1024] (fp32):
    xn  = LayerNorm(x) * ln_w + ln_b          (eps = 1e-5)
    qkv = xn @ w_qkv.T ; split into q, k, v   (16 heads, dim 64)
    y   = softmax(q k^T / sqrt(64)) v  @ w_out.T

Sharding: 8 cores = 4 batches x 2 head-groups (8 heads each).
Each core computes a partial output for its batch (its 8 heads); the
host sums the two head-group partials per batch.

Device-side layout is D-major ("transposed"): the host passes x[b].T
contiguously; LayerNorm statistics are computed with ones-matmuls on
the tensor engine, and the LN scale/shift is folded into the QKV
weights (host-side) plus a rank-2 augmentation of the contraction.
All matmuls run as float32r (full PE rate at moving-dim >= 256).
"""

import os
import sys

for _p in ("/opt/trn_rl_repo",):
    if _p not in sys.path and os.path.isdir(_p):
        sys.path.insert(0, _p)

import ml_dtypes
import numpy as np

import concourse.bass as bass
import concourse.tile as tile
from concourse import bacc, mybir
from concourse.bass_utils import run_bass_kernel_spmd

F32 = mybir.dt.float32
F32R = mybir.dt.float32r
BF16 = mybir.dt.bfloat16

B = 4            # batch
T = 2048         # sequence length
D = 1024         # model dim
HEADS_CORE = 8   # heads per core
DH = 64          # head dim
QK_OUT = 1024    # per-core q(512) + k(512) projection dims
V_OUT = 512      # per-core v dims
KT = D // 128    # 8 contraction tiles over D
N_CORES = 8
EPS = 1e-5


def build_program(loop_k: int = 0):
    """Build the per-core Bass program. loop_k>0 wraps the body in a
    hardware For_i loop (used only for timing measurements)."""
    nc = bacc.Bacc("TRN2", target_bir_lowering=False, debug=False,
                   num_devices=N_CORES)

    xT_d = nc.dram_tensor("xT", [D, T], F32R, kind="ExternalInput").ap()
    wqkT_d = nc.dram_tensor("wqkT", [D, QK_OUT], F32R, kind="ExternalInput").ap()
    wqk_aug_d = nc.dram_tensor("wqk_aug", [2, QK_OUT], F32R, kind="ExternalInput").ap()
    wvT_d = nc.dram_tensor("wvT", [D, V_OUT], F32R, kind="ExternalInput").ap()
    wv_aug_d = nc.dram_tensor("wv_aug", [2, V_OUT], F32R, kind="ExternalInput").ap()
    woutT_d = nc.dram_tensor("woutT", [V_OUT, D], BF16, kind="ExternalInput").ap()
    ones_row_d = nc.dram_tensor("ones_row", [1, T], F32R, kind="ExternalInput").ap()
    yT_d = nc.dram_tensor("yT", [D, T], F32, kind="ExternalOutput").ap()

    with tile.TileContext(nc) as tc:
        import contextlib
        ctx = contextlib.ExitStack()
        with ctx:
            persist = ctx.enter_context(tc.tile_pool(name="persist", bufs=1))
            qk_pool = ctx.enter_context(tc.tile_pool(name="qkp", bufs=4))
            w_pool = ctx.enter_context(tc.tile_pool(name="wp", bufs=10))
            wv_pool = ctx.enter_context(tc.tile_pool(name="wvp", bufs=3))
            wo_pool = ctx.enter_context(tc.tile_pool(name="wop", bufs=8))
            x2_pool = ctx.enter_context(tc.tile_pool(name="x2p", bufs=2))
            exp_pool = ctx.enter_context(tc.tile_pool(name="expp", bufs=2))
            ye_pool = ctx.enter_context(tc.tile_pool(name="yep", bufs=2))
            row_pool = ctx.enter_context(tc.tile_pool(name="rowp", bufs=2))
            stat_pool = ctx.enter_context(tc.tile_pool(name="statp", bufs=2))
            bc_pool = ctx.enter_context(tc.tile_pool(name="bcp", bufs=2))
            psB = ctx.enter_context(tc.tile_pool(name="psB", bufs=3, space="PSUM"))
            psU = ctx.enter_context(tc.tile_pool(name="psU", bufs=2, space="PSUM"))

            def body(_iter=0):
                # ---- persistent tiles -------------------------------
                xz = [persist.tile([128, T], F32R, tag=f"xz{kt}", name=f"xz{kt}") for kt in range(KT)]
                for kt in range(KT):
                    nc.sync.dma_start(xz[kt][:], xT_d[kt * 128:(kt + 1) * 128, :])

                wqk_aug_sb = persist.tile([2, QK_OUT], F32R, tag="wqkaug", name="wqkaug")
                nc.sync.dma_start(wqk_aug_sb[:], wqk_aug_d[:])
                wv_aug_sb = persist.tile([2, V_OUT], F32R, tag="wvaug", name="wvaug")
                nc.sync.dma_start(wv_aug_sb[:], wv_aug_d[:])

                ones_f = persist.tile([128, 1], F32, tag="ones_f", name="ones_f")
                nc.vector.memset(ones_f[:], 1.0)
                ones_r = persist.tile([128, 1], F32R, tag="ones_r", name="ones_r")
                nc.vector.tensor_copy(out=ones_r[:], in_=ones_f[:])
                ones8_f = persist.tile([128, 8], F32, tag="ones8", name="ones8")
                nc.vector.memset(ones8_f[:], 1.0)
                eps_t = persist.tile([1, 1], F32, tag="eps", name="eps")
                nc.vector.memset(eps_t[:], EPS)

                aug_z = persist.tile([2, T], F32R, tag="augz", name="augz")
                nc.sync.dma_start(aug_z[1:2, :], ones_row_d[:])
                rstd_bc = persist.tile([128, T], F32, tag="rstdbc", name="rstdbc")

                # ---- LN statistics: sum(x), sum(x^2) over D ---------
                inv_d = 1.0 / D
                for ih in range(2):
                  for c in range(2):
                    off = ih * 1024 + c * 512
                    pstat = psB.tile([128, 1024], F32, tag="big", name="big")
                    for kt in range(KT):
                        xs = xz[kt][:, off:off + 512]
                        nc.tensor.matmul(pstat[0:1, 0:512],
                                         ones_r[:], xs,
                                         start=(kt == 0), stop=(kt == KT - 1))
                        x2t = x2_pool.tile([128, 512], F32R, tag="x2", name="x2")
                        nc.vector.tensor_mul(x2t[:], xs, xs)
                        nc.tensor.matmul(pstat[0:1, 512:1024],
                                         ones_r[:], x2t[:],
                                         start=(kt == 0), stop=(kt == KT - 1))
                    # stats math per 512-chunk, all tiles at partition 0
                    if True:
                        gsl = slice(off, off + 512)
                        m = stat_pool.tile([1, 512], F32, tag="mrow", name="mrow")
                        nc.vector.tensor_copy(out=m[:], in_=pstat[0:1, 0:512])
                        v = stat_pool.tile([1, 512], F32, tag="vrow", name="vrow")
                        nc.vector.tensor_copy(out=v[:], in_=pstat[0:1, 512:1024])
                        nc.vector.tensor_scalar_mul(m[:], m[:], inv_d)
                        nc.vector.tensor_scalar_mul(v[:], v[:], inv_d)
                        s2 = stat_pool.tile([1, 512], F32, tag="srow2", name="srow2")
                        nc.vector.tensor_mul(s2[:], m[:], m[:])
                        nc.vector.tensor_sub(v[:], v[:], s2[:])
                        nc.scalar.activation(v[:], v[:],
                                             mybir.ActivationFunctionType.Sqrt,
                                             bias=eps_t[:])
                        r = stat_pool.tile([1, 512], F32, tag="rrow", name="rrow")
                        nc.vector.reciprocal_approx_accurate(r[:], v[:], s2[:])
                        nc.vector.tensor_mul(aug_z[0:1, gsl], m[:], r[:])  # mr
                        nc.gpsimd.partition_broadcast(rstd_bc[:, gsl], r[:])

                # z = x * rstd
                for kt in range(KT):
                    nc.vector.tensor_mul(xz[kt][:], xz[kt][:], rstd_bc[:])

                # ---- V projection (token-major) + ones column -------
                va = []
                for mt in range(16):
                    pv = psU.tile([128, 512], F32, tag="u", name="pv")
                    msl = slice(mt * 128, (mt + 1) * 128)
                    for kt in range(KT):
                        wv_t = wv_pool.tile([128, V_OUT], F32R, tag="wv", name="wv")
                        nc.sync.dma_start(wv_t[:], wvT_d[kt * 128:(kt + 1) * 128, :])
                        nc.tensor.matmul(pv[:], xz[kt][:, msl], wv_t[:],
                                         start=(kt == 0), stop=False)
                    nc.tensor.matmul(pv[:], aug_z[:, msl], wv_aug_sb[:],
                                     start=False, stop=True)
                    va_t = persist.tile([128, 8, 65], BF16, tag=f"va{mt}", name=f"va{mt}")
                    nc.vector.tensor_copy(
                        out=va_t[:, :, 0:64],
                        in_=pv[:].rearrange("p (h e) -> p h e", h=8))
                    nc.vector.tensor_copy(out=va_t[:, :, 64], in_=ones8_f[:])
                    va.append(va_t)

                # ---- per-pair QK projection + attention -------------
                attn_out = [persist.tile([128, T], BF16, tag=f"ao{p}", name=f"ao{p}")
                            for p in range(4)]
                yseen = []
                for p in range(4):
                    qk = {}
                    for mt in (p, 4 + p):
                        wts = []
                        for kt in range(KT):
                            wt = w_pool.tile([128, 128], F32R, tag="wqk", name="wqk")
                            nc.sync.dma_start(
                                wt[:], wqkT_d[kt * 128:(kt + 1) * 128,
                                              mt * 128:(mt + 1) * 128])
                            wts.append(wt)
                        qkt = qk_pool.tile([128, T], F32R, tag="qk", name="qk")
                        qk[mt] = qkt
                        for ih in range(2):
                            pq = psB.tile([128, 1024], F32, tag="big", name="big")
                            for kt in range(KT):
                                for c in range(2):
                                    off = ih * 1024 + c * 512
                                    nc.tensor.matmul(
                                        pq[:, c * 512:(c + 1) * 512],
                                        wts[kt][:], xz[kt][:, off:off + 512],
                                        start=(kt == 0), stop=False)
                            for c in range(2):
                                off = ih * 1024 + c * 512
                                nc.tensor.matmul(
                                    pq[:, c * 512:(c + 1) * 512],
                                    wqk_aug_sb[:, mt * 128:(mt + 1) * 128],
                                    aug_z[:, off:off + 512],
                                    start=False, stop=True)
                            nc.vector.tensor_copy(
                                out=qkt[:, ih * 1024:(ih + 1) * 1024], in_=pq[:])

                    qt, kt_t = qk[p], qk[4 + p]
                    for ic in range(4):
                        isl = slice(ic * 512, (ic + 1) * 512)
                        u01 = [psU.tile([65, 512], F32, tag="u", name="u") for _ in range(2)]
                        for jt in range(16):
                            jsl = slice(jt * 128, (jt + 1) * 128)
                            pd = psB.tile([128, 1024], F32, tag="big", name="big")
                            nc.tensor.matmul(pd[:, 0:512],
                                             kt_t[0:64, jsl], qt[0:64, isl],
                                             start=True, stop=True)
                            nc.tensor.matmul(pd[:, 512:1024],
                                             kt_t[64:128, jsl], qt[64:128, isl],
                                             start=True, stop=True)
                            et = exp_pool.tile([128, 1024], BF16, tag="exp", name="exp")
                            nc.scalar.activation(
                                et[:], pd[:], mybir.ActivationFunctionType.Exp)
                            nc.tensor.matmul(u01[0][:], va[jt][:, 2 * p, :],
                                             et[:, 0:512],
                                             start=(jt == 0), stop=(jt == 15))
                            nc.tensor.matmul(u01[1][:], va[jt][:, 2 * p + 1, :],
                                             et[:, 512:1024],
                                             start=(jt == 0), stop=(jt == 15))
                        for h01 in range(2):
                            u = u01[h01]
                            s_t = row_pool.tile([1, 512], F32, tag="stmp", name="stmp")
                            nc.vector.tensor_copy(out=s_t[:], in_=u[64:65, :])
                            r_t = row_pool.tile([1, 512], F32, tag="rtmp", name="rtmp")
                            nc.vector.reciprocal_approx_fast(r_t[:], s_t[:])
                            bc = bc_pool.tile([64, 512], F32, tag="bc", name="bc")
                            nc.gpsimd.partition_broadcast(bc[:], r_t[:])
                            nc.vector.tensor_mul(
                                attn_out[p][h01 * 64:(h01 + 1) * 64, isl],
                                u[0:64, :], bc[:])

                # ---- output projection ------------------------------
                for mt in range(8):
                    wos = []
                    for kt in range(4):
                        wo = wo_pool.tile([128, 128], BF16, tag="wo", name="wo")
                        nc.sync.dma_start(
                            wo[:], woutT_d[kt * 128:(kt + 1) * 128,
                                           mt * 128:(mt + 1) * 128])
                        wos.append(wo)
                    for ih in range(2):
                        py = psB.tile([128, 1024], F32, tag="big", name="big")
                        for kt in range(4):
                            for c in range(2):
                                off = ih * 1024 + c * 512
                                nc.tensor.matmul(
                                    py[:, c * 512:(c + 1) * 512],
                                    wos[kt][:], attn_out[kt][:, off:off + 512],
                                    start=(kt == 0), stop=(kt == 3))
                        ye = ye_pool.tile([128, 1024], F32, tag="ye", name="ye")
                        nc.vector.tensor_copy(out=ye[:], in_=py[:])
                        nc.sync.dma_start(
                            yT_d[mt * 128:(mt + 1) * 128,
                                 ih * 1024:(ih + 1) * 1024], ye[:])

            if loop_k and loop_k > 1:
                with tc.For_i(0, loop_k, 1):
                    body()
            else:
                body()

    nc.compile()
    return nc


def _prep_inputs(x, ln_w, ln_b, w_qkv, w_out):
    """Host-side sharding + LN/scale folding. Returns per-core in_maps."""
    x = np.asarray(x, dtype=np.float32)
    ln_w = np.asarray(ln_w, dtype=np.float32)
    ln_b = np.asarray(ln_b, dtype=np.float32)
    w_qkv = np.asarray(w_qkv, dtype=np.float32)
    w_out = np.asarray(w_out, dtype=np.float32)

    scale = DH ** -0.5
    in_maps = []
    xT_cache = {}
    for c in range(N_CORES):
        b, g = c // 2, c % 2
        if b not in xT_cache:
            xT_cache[b] = np.ascontiguousarray(x[b].T)
        gs = slice(g * 512, (g + 1) * 512)
        wq = w_qkv[0:1024][gs] * scale
        wk = w_qkv[1024:2048][gs]
        wv = w_qkv[2048:3072][gs]

        def fold(w_eff):
            wp = w_eff * ln_w[None, :]
            c1 = -wp.sum(axis=1)
            c2 = w_eff @ ln_b
            return wp, c1, c2

        wqp, c1q, c2q = fold(wq)
        wkp, c1k, c2k = fold(wk)
        wvp, c1v, c2v = fold(wv)

        wqk = np.concatenate([wqp, wkp], axis=0)          # [1024, 1024]
        wqk_aug = np.stack([np.concatenate([c1q, c1k]),
                            np.concatenate([c2q, c2k])])   # [2, 1024]
        wv_aug = np.stack([c1v, c2v])                      # [2, 512]

        in_maps.append({
            "xT": xT_cache[b],
            "ones_row": np.ones((1, T), dtype=np.float32),
            "wqkT": np.ascontiguousarray(wqk.T),
            "wqk_aug": np.ascontiguousarray(wqk_aug),
            "wvT": np.ascontiguousarray(wvp.T),
            "wv_aug": np.ascontiguousarray(wv_aug),
            "woutT": np.ascontiguousarray(
                w_out[:, g * 512:(g + 1) * 512].T).astype(ml_dtypes.bfloat16),
        })
    return in_maps


_PROG = None


def kernel(x, ln_w, ln_b, w_qkv, w_out):
    global _PROG
    if _PROG is None:
        _PROG = build_program()
    nc = _PROG
    in_maps = _prep_inputs(x, ln_w, ln_b, w_qkv, w_out)
    res = run_bass_kernel_spmd(nc, in_maps, core_ids=list(range(N_CORES)))
    y = np.empty((B, T, D), dtype=np.float32)
    for b in range(B):
        yT = res.results[2 * b]["yT"] + res.results[2 * b + 1]["yT"]
        y[b] = yT.T
    return y
